# revision 11
# baseline (speedup 1.0000x reference)
"""LDA-loss logits kernel for Trainium2 (8 NeuronCores, SPMD).

Computes logits[b, c] = -0.5 * ||feat[b] - centers[c]||^2
                      = feat[b]·centers[c] - 0.5||feat[b]||^2 - 0.5||centers[c]||^2

Strategy (v8, fp8 DoubleRow, 2x4 hybrid shard):
  - 2 batch halves x 4 class quarters: each core computes 2048 rows x 2560
    classes (2500 real + 60 pad = exactly 5 n-tiles of 512).  Same 320-matmul
    instruction stream as pure batch sharding, but input HBM traffic drops
    from 13.6MB to 5.3MB/core, so the in+out DMA demand (~16MB over the 73us
    matmul stream) stays well under the ~358 GB/s per-core HBM limit -- the
    pure batch-shard version ran at ~330 GB/s and suffered input-starvation
    stalls mid-stream.
  - Inputs quantized to fp8e4 on host; matmuls run perf_mode=DoubleRow
    (2 contraction chunks of 128 per pass, ~96% of the 157 TF/s fp8 peak).
    Squared-norm biases are host-precomputed fp32/fp16; fp8 error only
    touches the cross term (~6e-3 rel, tolerance 2e-2).
  - Loop: m-group outer (4 groups of 4 m-tiles), n-tile inner (5), k-pair
    innermost; centers stay SBUF-resident and are re-read per m-group.
    First loads split fine-grained so the first matmul issues after ~256KB.
  - ~6us of warm-up matmuls during the DMA prologue open the HAM clock gate
    (1.2 -> 2.4 GHz) before the first real matmul.
  - Eviction: ScalarE adds the per-row bias (fp32 PSUM -> fp16 SBUF),
    VectorE adds the per-column bias (fp16).  Host upcasts the fp16 output
    after the gather.  Input loads ride the sync HWDGE ring; output stores
    ride the Activation HWDGE ring (separate FIFOs -- stores behind loads on
    one ring deadlocks the eviction pipeline; SWDGE stores pay a multi-us
    drain).  The final n-tile stores per m-tile from the idle sync ring so
    the last store pipelines with the remaining evictions.
"""

import numpy as np
import ml_dtypes

BATCH = 4096
FEAT_DIM = 1024
NUM_CLASSES = 10000
N_CORES = 8
BQ = 2                              # batch shards
CQ = 4                              # class shards
B_PER = BATCH // BQ                 # 2048 rows per core
P = 128
MT = B_PER // P                     # 16 m-tiles per core
MG = 4                              # m-groups of 4 m-tiles
KO = FEAT_DIM // P                  # 8 contraction chunks
KP = KO // 2                        # 4 DoubleRow chunk-pairs
NT = 5                              # n-tiles of 512 per core
C_PER = NT * 512                    # 2560 padded classes per core
C_REAL = NUM_CLASSES // CQ          # 2500 real classes per core

_NC = None


def _build_bass():
    import concourse.mybir as mybir
    import concourse.tile as tile
    from concourse import bacc

    nc = bacc.Bacc("TRN2", target_bir_lowering=False, debug=False)

    featT = nc.dram_tensor("featT", [MT, P, KO * P], mybir.dt.float8e4,
                           kind="ExternalInput")
    centsT = nc.dram_tensor("centsT", [NT, P, KO * 512], mybir.dt.float8e4,
                            kind="ExternalInput")
    fsq = nc.dram_tensor("fsq", [P, MT], mybir.dt.float32, kind="ExternalInput")
    csq = nc.dram_tensor("csq", [P, NT * 512], mybir.dt.float16,
                         kind="ExternalInput")
    out = nc.dram_tensor("out", [B_PER, C_PER], mybir.dt.float16,
                         kind="ExternalOutput")

    with tile.TileContext(nc) as tc:
        _lda_tile_kernel(tc, featT.ap(), centsT.ap(), fsq.ap(), csq.ap(),
                         out.ap())
    nc.compile()
    return nc


def _lda_tile_kernel(tc, featT, centsT, fsq, csq, out):
    import concourse.mybir as mybir

    nc = tc.nc
    out_r = out.rearrange("(mo p) c -> p mo c", p=P)
    featT_r = featT.rearrange("mt p f -> p mt f")

    with (
        tc.tile_pool(name="big", bufs=1) as big,
        tc.tile_pool(name="consts", bufs=1) as consts,
        tc.tile_pool(name="ostage", bufs=6) as ostage,
        tc.tile_pool(name="psum", bufs=8, space="PSUM") as psum,
    ):
        cent_sb = big.tile([P, NT, KO, 512], mybir.dt.float8e4)
        feat_sb = big.tile([P, MT, KO, P], mybir.dt.float8e4)
        csq_sb = consts.tile([P, NT, 512], mybir.dt.float16)
        fsq_sb = consts.tile([P, MT], mybir.dt.float32)
        warm_sb = consts.tile([P, 192], mybir.dt.float8e4)

        # Input loads on the sync HWDGE ring in consumption order.  The
        # first matmul needs only feat m-tile 0 + the first k-pair of center
        # n-tile 0 (~256KB), so those are split fine-grained.
        c0 = centsT[0].rearrange("p (ko c) -> p ko c", ko=KO)
        nc.sync.dma_start(feat_sb[:, 0:1],
                          featT_r[:, 0:1].rearrange("p m (ko f) -> p m ko f",
                                                    ko=KO))
        nc.sync.dma_start(cent_sb[:, 0, 0:2], c0[:, 0:2])
        nc.sync.dma_start(feat_sb[:, 1:MG],
                          featT_r[:, 1:MG].rearrange("p m (ko f) -> p m ko f",
                                                     ko=KO))
        for kp in range(1, KP):
            nc.sync.dma_start(cent_sb[:, 0, 2 * kp:2 * kp + 2],
                              c0[:, 2 * kp:2 * kp + 2])
        nc.sync.dma_start(fsq_sb[:], fsq)
        nc.sync.dma_start(csq_sb[:], csq.rearrange("p (j c) -> p j c", j=NT))
        for j in range(1, NT):
            nc.sync.dma_start(cent_sb[:, j],
                              centsT[j].rearrange("p (ko c) -> p ko c", ko=KO))
        for mg in range(1, MG):
            nc.sync.dma_start(
                feat_sb[:, 4 * mg:4 * mg + 4],
                featT_r[:, 4 * mg:4 * mg + 4].rearrange(
                    "p m (ko f) -> p m ko f", ko=KO))

        # PE warm-up: ~6us of throwaway matmuls during the DMA prologue so
        # the HAM clock gate opens (1.2 -> 2.4 GHz takes ~3.4us of sustained
        # PE activity) before the first real matmul issues.
        nc.vector.memset(warm_sb[:], 0)
        warm_ps = psum.tile([P, 512], mybir.dt.float32, tag="ps", name="ps")
        for _ in range(56):
            nc.tensor.matmul(warm_ps[:, 0:64], warm_sb[:, 0:P],
                             warm_sb[:, P:P + 64], start=True, stop=True)

        for mg in range(MG):
            for j in range(NT):
                last = (mg == MG - 1 and j == NT - 1)
                ps = [psum.tile([P, 512], mybir.dt.float32, tag="ps",
                                name="ps") for _ in range(4)]
                for kp in range(KP):
                    for ml in range(4):
                        nc.tensor.matmul(
                            ps[ml],
                            feat_sb[:, 4 * mg + ml, 2 * kp:2 * kp + 2, :],
                            cent_sb[:, j, 2 * kp:2 * kp + 2, :],
                            start=(kp == 0),
                            stop=(kp == KP - 1),
                            perf_mode=mybir.MatmulPerfMode.DoubleRow,
                        )
                ot = ostage.tile([P, 4, 512], mybir.dt.float16, tag="ot",
                                 name="ot")
                for ml in range(4):
                    m = 4 * mg + ml
                    # ot[ml] = psum + fsq[row]  (per-partition bias, ScalarE)
                    nc.scalar.activation(
                        ot[:, ml], ps[ml],
                        mybir.ActivationFunctionType.Identity,
                        bias=fsq_sb[:, m:m + 1],
                    )
                    # ot[ml] += csq[col]  (per-column bias, VectorE, fp16)
                    nc.vector.tensor_add(ot[:, ml], ot[:, ml], csq_sb[:, j])
                    if last:
                        # Final n-tile: store per m-tile from the sync ring
                        # (idle by now) so the last store pipelines with the
                        # remaining evictions.
                        nc.sync.dma_start(
                            out_r[:, m, j * 512:(j + 1) * 512], ot[:, ml])
                if not last:
                    # Stores on the second HWDGE ring (Activation engine):
                    # separate FIFO from the input ring, so stores never
                    # queue behind loads.
                    nc.scalar.dma_start(
                        out_r[:, 4 * mg:4 * mg + 4, j * 512:(j + 1) * 512],
                        ot)


def _get_nc():
    global _NC
    if _NC is None:
        _NC = _build_bass()
    return _NC


def _prep_inputs(feat, centers):
    feat = np.asarray(feat, dtype=np.float32)
    centers = np.asarray(centers, dtype=np.float32)
    f8 = ml_dtypes.float8_e4m3

    feat8 = feat.astype(f8)
    fsq_v = -0.5 * np.einsum("bd,bd->b", feat, feat)

    feat_maps = []
    for b in range(BQ):
        r0 = b * B_PER
        # featT_sw[mt, p, ko*128 + m] = feat[r0 + mt*128 + m, ko*128 + p]
        featT_sw = np.ascontiguousarray(
            feat8[r0:r0 + B_PER].T.reshape(KO, P, MT, P).transpose(2, 1, 0, 3)
        ).reshape(MT, P, KO * P)
        fsq_mat = np.ascontiguousarray(
            fsq_v[r0:r0 + B_PER].reshape(MT, P).T)
        feat_maps.append((featT_sw, fsq_mat))

    cent_maps = []
    for q in range(CQ):
        c0 = q * C_REAL
        cent_pad = np.zeros((C_PER, FEAT_DIM), dtype=np.float32)
        cent_pad[:C_REAL] = centers[c0:c0 + C_REAL]
        # centsT_sw[j, p, ko*512 + c] = cent_pad[j*512 + c, ko*128 + p]
        centsT_sw = np.ascontiguousarray(
            cent_pad.T.astype(f8).reshape(KO, P, NT, 512).transpose(2, 1, 0, 3)
        ).reshape(NT, P, KO * 512)
        csq_v = np.zeros(C_PER, dtype=np.float32)
        csq_v[:C_REAL] = -0.5 * np.einsum(
            "cd,cd->c", cent_pad[:C_REAL], cent_pad[:C_REAL])
        csq_sw = np.ascontiguousarray(np.broadcast_to(
            csq_v.astype(np.float16)[None, :], (P, C_PER)))
        cent_maps.append((centsT_sw, csq_sw))

    in_maps = []
    for i in range(N_CORES):
        featT_sw, fsq_mat = feat_maps[i // CQ]
        centsT_sw, csq_sw = cent_maps[i % CQ]
        in_maps.append({
            "featT": featT_sw,
            "centsT": centsT_sw,
            "fsq": fsq_mat,
            "csq": csq_sw,
        })
    return in_maps


def _run(inputs, trace=False, trace_cores=None):
    from concourse import bass_utils

    nc = _get_nc()
    in_maps = _prep_inputs(inputs["feat"], inputs["centers"])
    res = bass_utils.run_bass_kernel_spmd(
        nc, in_maps, core_ids=list(range(N_CORES)), trace=trace,
        trace_cores=trace_cores,
    )
    rows = []
    for b in range(BQ):
        rows.append(np.concatenate(
            [np.asarray(res.results[b * CQ + q]["out"])[:, :C_REAL]
             for q in range(CQ)], axis=1))
    return np.concatenate(rows, axis=0).astype(np.float32), res


def kernel(**inputs) -> np.ndarray:
    return _run(inputs)[0]


# revision 13
# speedup vs baseline: 1.0098x; 1.0098x over previous
"""LDA-loss logits kernel for Trainium2 (8 NeuronCores, SPMD).

Computes logits[b, c] = -0.5 * ||feat[b] - centers[c]||^2
                      = feat[b]·centers[c] - 0.5||feat[b]||^2 - 0.5||centers[c]||^2

Strategy (v3, fp8 DoubleRow):
  - Shard feat over batch: 4096 rows -> 512/core (4 m-tiles of 128), centers
    replicated.  Classes padded 10000 -> 10240 = 20 n-tiles of 512 so every
    matmul streams a full 512-wide moving operand (hides LDWEIGHTS).
  - Inputs quantized to fp8e4 on host; matmuls run perf_mode=DoubleRow
    (2 contraction chunks of 128 per pass -> ~1.8x bf16 column rate).  The
    squared-norm biases are host-precomputed fp32/fp16; fp8 error only
    touches the cross term (~6e-3 rel, tolerance 2e-2).
  - n-outer loop; center n-tiles stream in individually, and the first
    n-tile/feat are further split (per k-pair / per m-tile) so the first
    matmul issues after ~256KB of DMA instead of the full working set.
  - Eviction: ScalarE adds the per-row bias (fp32 PSUM -> fp16 SBUF),
    VectorE adds the per-column bias (fp16, 2x DVE rate).  The host upcasts
    the fp16 output after the gather (halves output HBM traffic).
  - All DMA on the sync HWDGE queue (SWDGE/gpsimd pays ~2us completion
    latency per store and a multi-us drain on the final ack); outputs are
    batched 4 m-tiles per store so the queue holds 20 output DMAs, not 80.
"""

import numpy as np
import ml_dtypes

BATCH = 4096
FEAT_DIM = 1024
NUM_CLASSES = 10000
N_CORES = 8
B_PER = BATCH // N_CORES            # 512 rows per core
P = 128
MT = B_PER // P                     # 4 output row tiles per core
KO = FEAT_DIM // P                  # 8 contraction chunks
KP = KO // 2                        # 4 DoubleRow chunk-pairs
NT = 20                             # n-tiles of 512
C_PAD = NT * 512                    # 10240 padded classes

_NC = None


def _build_bass():
    import concourse.mybir as mybir
    import concourse.tile as tile
    from concourse import bacc

    nc = bacc.Bacc("TRN2", target_bir_lowering=False, debug=False)

    featT = nc.dram_tensor("featT", [MT, P, KO * P], mybir.dt.float8e4,
                           kind="ExternalInput")
    centsT = nc.dram_tensor("centsT", [NT, P, KO * 512], mybir.dt.float8e4,
                            kind="ExternalInput")
    fsq = nc.dram_tensor("fsq", [P, MT], mybir.dt.float32, kind="ExternalInput")
    csq = nc.dram_tensor("csq", [NT // 4, P, 4 * 512], mybir.dt.float16,
                         kind="ExternalInput")
    out = nc.dram_tensor("out", [B_PER, C_PAD], mybir.dt.float16,
                         kind="ExternalOutput")

    with tile.TileContext(nc) as tc:
        _lda_tile_kernel(tc, featT.ap(), centsT.ap(), fsq.ap(), csq.ap(),
                         out.ap())
    nc.compile()
    return nc


def _lda_tile_kernel(tc, featT, centsT, fsq, csq, out):
    import concourse.mybir as mybir

    nc = tc.nc
    out_r = out.rearrange("(mo p) c -> p mo c", p=P)

    with (
        tc.tile_pool(name="big", bufs=1) as big,
        tc.tile_pool(name="consts", bufs=1) as consts,
        tc.tile_pool(name="ostage", bufs=6) as ostage,
        tc.tile_pool(name="psum", bufs=8, space="PSUM") as psum,
    ):
        cent_sb = big.tile([P, NT, KO, 512], mybir.dt.float8e4)
        feat_sb = big.tile([P, MT, KO, P], mybir.dt.float8e4)
        csq_sb = consts.tile([P, NT, 512], mybir.dt.float16)
        fsq_sb = consts.tile([P, MT], mybir.dt.float32)
        warm_sb = consts.tile([P, 192], mybir.dt.float8e4)

        # All input loads on the sync HWDGE queue in consumption order.  The
        # first matmul needs only feat m-tile 0 + the first k-pair of center
        # n-tile 0 (~256KB), so split those loads fine-grained; everything
        # later goes in n-tile-sized chunks that stay ahead of compute.
        c0 = centsT[0].rearrange("p (ko c) -> p ko c", ko=KO)
        nc.sync.dma_start(feat_sb[:, 0],
                          featT[0].rearrange("p (ko f) -> p ko f", ko=KO))
        nc.sync.dma_start(cent_sb[:, 0, 0:2], c0[:, 0:2])
        for m in range(1, MT):
            nc.sync.dma_start(
                feat_sb[:, m], featT[m].rearrange("p (ko f) -> p ko f", ko=KO))
        for kp in range(1, KP):
            nc.sync.dma_start(cent_sb[:, 0, 2 * kp:2 * kp + 2],
                              c0[:, 2 * kp:2 * kp + 2])
        nc.sync.dma_start(fsq_sb[:], fsq)
        for j in range(1, NT):
            nc.sync.dma_start(cent_sb[:, j],
                              centsT[j].rearrange("p (ko c) -> p ko c", ko=KO))
            if j % 4 == 1:
                b = j // 4
                nc.sync.dma_start(
                    csq_sb[:, 4 * b:4 * b + 4],
                    csq[b].rearrange("p (j c) -> p j c", j=4))

        # PE warm-up: ~6us of throwaway matmuls during the DMA prologue so
        # the HAM clock gate opens (1.2 -> 2.4 GHz takes ~3.4us of sustained
        # PE activity) before the first real matmul issues.
        nc.vector.memset(warm_sb[:], 0)
        warm_ps = psum.tile([P, 512], mybir.dt.float32, tag="ps", name="ps")
        for _ in range(56):
            nc.tensor.matmul(warm_ps[:, 0:64], warm_sb[:, 0:P],
                             warm_sb[:, P:P + 64], start=True, stop=True)

        for j in range(NT):
            ps = [psum.tile([P, 512], mybir.dt.float32, tag="ps", name="ps")
                  for _ in range(MT)]
            for kp in range(KP):
                for m in range(MT):
                    nc.tensor.matmul(
                        ps[m],
                        feat_sb[:, m, 2 * kp:2 * kp + 2, :],
                        cent_sb[:, j, 2 * kp:2 * kp + 2, :],
                        start=(kp == 0),
                        stop=(kp == KP - 1),
                        perf_mode=mybir.MatmulPerfMode.DoubleRow,
                    )
            ot = ostage.tile([P, MT, 512], mybir.dt.float16, tag="ot",
                             name="ot")
            for m in range(MT):
                # ot[m] = psum + fsq[row]  (per-partition bias on ScalarE)
                nc.scalar.activation(
                    ot[:, m], ps[m], mybir.ActivationFunctionType.Identity,
                    bias=fsq_sb[:, m:m + 1],
                )
                # ot[m] += csq[col]  (per-column bias on VectorE, fp16)
                nc.vector.tensor_add(ot[:, m], ot[:, m], csq_sb[:, j])
                if j == NT - 1:
                    # Final n-tile: store per m-tile from the sync ring
                    # (idle by now) so the last store pipelines with the
                    # remaining evictions instead of waiting for all four.
                    nc.sync.dma_start(
                        out_r[:, m, j * 512:(j + 1) * 512], ot[:, m])
            if j < NT - 1:
                # Output on the second HWDGE ring (Activation engine):
                # separate FIFO from the input ring, so stores never queue
                # behind loads.
                nc.scalar.dma_start(out_r[:, :, j * 512:(j + 1) * 512], ot)


def _get_nc():
    global _NC
    if _NC is None:
        _NC = _build_bass()
    return _NC


def _prep_inputs(feat, centers):
    feat = np.asarray(feat, dtype=np.float32)
    centers = np.asarray(centers, dtype=np.float32)
    f8 = ml_dtypes.float8_e4m3

    cent_pad = np.zeros((C_PAD, FEAT_DIM), dtype=np.float32)
    cent_pad[:NUM_CLASSES] = centers
    # centsT_sw[j, p, ko*512 + c] = centers[j*512 + c, ko*128 + p]
    centsT_sw = np.ascontiguousarray(
        cent_pad.T.astype(f8).reshape(KO, P, NT, 512).transpose(2, 1, 0, 3)
    ).reshape(NT, P, KO * 512)

    csq_v = np.zeros(C_PAD, dtype=np.float32)
    csq_v[:NUM_CLASSES] = -0.5 * np.einsum("cd,cd->c", centers, centers)
    csq_sw = np.ascontiguousarray(np.broadcast_to(
        csq_v.astype(np.float16).reshape(NT // 4, 1, 4 * 512),
        (NT // 4, P, 4 * 512)))

    feat8 = feat.astype(f8)
    fsq_v = -0.5 * np.einsum("bd,bd->b", feat, feat)

    in_maps = []
    for i in range(N_CORES):
        r0 = i * B_PER
        # featT_sw[mt, p, ko*128 + m] = feat[r0 + mt*128 + m, ko*128 + p]
        featT_sw = np.ascontiguousarray(
            feat8[r0:r0 + B_PER].T.reshape(KO, P, MT, P).transpose(2, 1, 0, 3)
        ).reshape(MT, P, KO * P)
        fsq_mat = np.ascontiguousarray(
            fsq_v[r0:r0 + B_PER].reshape(MT, P).T)
        in_maps.append({
            "featT": featT_sw,
            "centsT": centsT_sw,
            "fsq": fsq_mat,
            "csq": csq_sw,
        })
    return in_maps


def _run(inputs, trace=False, trace_cores=None):
    from concourse import bass_utils

    nc = _get_nc()
    in_maps = _prep_inputs(inputs["feat"], inputs["centers"])
    res = bass_utils.run_bass_kernel_spmd(
        nc, in_maps, core_ids=list(range(N_CORES)), trace=trace,
        trace_cores=trace_cores,
    )
    full = np.concatenate(
        [np.asarray(res.results[i]["out"]) for i in range(N_CORES)], axis=0)
    return full[:, :NUM_CLASSES].astype(np.float32), res


def kernel(**inputs) -> np.ndarray:
    return _run(inputs)[0]


# revision 14
# speedup vs baseline: 1.0548x; 1.0446x over previous
"""LDA-loss logits kernel for Trainium2 (8 NeuronCores, SPMD).

Computes logits[b, c] = -0.5 * ||feat[b] - centers[c]||^2
                      = feat[b]·centers[c] - 0.5||feat[b]||^2 - 0.5||centers[c]||^2

Strategy (v3, fp8 DoubleRow):
  - Shard feat over batch: 4096 rows -> 512/core (4 m-tiles of 128), centers
    replicated.  Classes padded 10000 -> 10240 = 20 n-tiles of 512 so every
    matmul streams a full 512-wide moving operand (hides LDWEIGHTS).
  - Inputs quantized to fp8e4 on host; matmuls run perf_mode=DoubleRow
    (2 contraction chunks of 128 per pass -> ~1.8x bf16 column rate).  The
    squared-norm biases are host-precomputed fp32/fp16; fp8 error only
    touches the cross term (~6e-3 rel, tolerance 2e-2).
  - n-outer loop; center n-tiles stream in individually, and the first
    n-tile/feat are further split (per k-pair / per m-tile) so the first
    matmul issues after ~256KB of DMA instead of the full working set.
  - Eviction: ScalarE adds the per-row bias (fp32 PSUM -> fp16 SBUF),
    VectorE adds the per-column bias (fp16, 2x DVE rate).  The host upcasts
    the fp16 output after the gather (halves output HBM traffic).
  - All DMA on the sync HWDGE queue (SWDGE/gpsimd pays ~2us completion
    latency per store and a multi-us drain on the final ack); outputs are
    batched 4 m-tiles per store so the queue holds 20 output DMAs, not 80.
"""

import numpy as np
import ml_dtypes

BATCH = 4096
FEAT_DIM = 1024
NUM_CLASSES = 10000
N_CORES = 8
B_PER = BATCH // N_CORES            # 512 rows per core
P = 128
MT = B_PER // P                     # 4 output row tiles per core
KO = FEAT_DIM // P                  # 8 contraction chunks
KP = KO // 2                        # 4 DoubleRow chunk-pairs
NT = 20                             # n-tiles of 512
C_PAD = NT * 512                    # 10240 padded classes

_NC = None


def _build_bass():
    import concourse.mybir as mybir
    import concourse.tile as tile
    from concourse import bacc

    nc = bacc.Bacc("TRN2", target_bir_lowering=False, debug=False)

    featT = nc.dram_tensor("featT", [MT, P, KO * P], mybir.dt.float8e4,
                           kind="ExternalInput")
    centsT = nc.dram_tensor("centsT", [NT, P, KO * 512], mybir.dt.float8e4,
                            kind="ExternalInput")
    fsq = nc.dram_tensor("fsq", [P, MT], mybir.dt.float32, kind="ExternalInput")
    csq = nc.dram_tensor("csq", [NT // 4, P, 4 * 512], mybir.dt.float16,
                         kind="ExternalInput")
    out = nc.dram_tensor("out", [B_PER, C_PAD], mybir.dt.float16,
                         kind="ExternalOutput")

    with tile.TileContext(nc) as tc:
        _lda_tile_kernel(tc, featT.ap(), centsT.ap(), fsq.ap(), csq.ap(),
                         out.ap())
    nc.compile()
    return nc


def _lda_tile_kernel(tc, featT, centsT, fsq, csq, out):
    import concourse.mybir as mybir

    nc = tc.nc
    out_r = out.rearrange("(mo p) c -> p mo c", p=P)

    with (
        tc.tile_pool(name="big", bufs=1) as big,
        tc.tile_pool(name="consts", bufs=1) as consts,
        tc.tile_pool(name="ostage", bufs=6) as ostage,
        tc.tile_pool(name="psum", bufs=8, space="PSUM") as psum,
    ):
        cent_sb = big.tile([P, NT, KO, 512], mybir.dt.float8e4)
        feat_sb = big.tile([P, MT, KO, P], mybir.dt.float8e4)
        csq_sb = consts.tile([P, NT, 512], mybir.dt.float16)
        fsq_sb = consts.tile([P, MT], mybir.dt.float32)
        warm_sb = consts.tile([P, 192], mybir.dt.float8e4)

        # All input loads on the sync HWDGE queue in consumption order.  The
        # first matmul needs only feat m-tile 0 + the first k-pair of center
        # n-tile 0 (~256KB), so split those loads fine-grained; everything
        # later goes in n-tile-sized chunks that stay ahead of compute.
        c0 = centsT[0].rearrange("p (ko c) -> p ko c", ko=KO)
        nc.sync.dma_start(feat_sb[:, 0],
                          featT[0].rearrange("p (ko f) -> p ko f", ko=KO))
        nc.sync.dma_start(cent_sb[:, 0, 0:2], c0[:, 0:2])
        for m in range(1, MT):
            nc.sync.dma_start(
                feat_sb[:, m], featT[m].rearrange("p (ko f) -> p ko f", ko=KO))
        for kp in range(1, KP):
            nc.sync.dma_start(cent_sb[:, 0, 2 * kp:2 * kp + 2],
                              c0[:, 2 * kp:2 * kp + 2])
        nc.sync.dma_start(fsq_sb[:], fsq)
        for j in range(1, 4):
            nc.sync.dma_start(cent_sb[:, j],
                              centsT[j].rearrange("p (ko c) -> p ko c", ko=KO))
            if j == 1:
                nc.sync.dma_start(csq_sb[:, 0:4],
                                  csq[0].rearrange("p (j c) -> p j c", j=4))
        # Input runs ~2.7x ahead of consumption from here on, so later
        # center tiles load in 4-tile batches: fewer serial ~633ns
        # descriptor-gen dispatches and fewer per-tile semaphores for the
        # matmul stream to check.
        for b in range(1, NT // 4):
            nc.sync.dma_start(
                cent_sb[:, 4 * b:4 * b + 4],
                centsT[4 * b:4 * b + 4].rearrange(
                    "n p (ko c) -> p n ko c", ko=KO))
            nc.sync.dma_start(
                csq_sb[:, 4 * b:4 * b + 4],
                csq[b].rearrange("p (j c) -> p j c", j=4))

        # PE warm-up: ~6us of throwaway matmuls during the DMA prologue so
        # the HAM clock gate opens (1.2 -> 2.4 GHz takes ~3.4us of sustained
        # PE activity) before the first real matmul issues.
        nc.vector.memset(warm_sb[:], 0)
        warm_ps = psum.tile([P, 512], mybir.dt.float32, tag="ps", name="ps")
        for _ in range(56):
            nc.tensor.matmul(warm_ps[:, 0:64], warm_sb[:, 0:P],
                             warm_sb[:, P:P + 64], start=True, stop=True)

        for j in range(NT):
            ps = [psum.tile([P, 512], mybir.dt.float32, tag="ps", name="ps")
                  for _ in range(MT)]
            for kp in range(KP):
                for m in range(MT):
                    nc.tensor.matmul(
                        ps[m],
                        feat_sb[:, m, 2 * kp:2 * kp + 2, :],
                        cent_sb[:, j, 2 * kp:2 * kp + 2, :],
                        start=(kp == 0),
                        stop=(kp == KP - 1),
                        perf_mode=mybir.MatmulPerfMode.DoubleRow,
                    )
            ot = ostage.tile([P, MT, 512], mybir.dt.float16, tag="ot",
                             name="ot")
            for m in range(MT):
                # ot[m] = psum + fsq[row]  (per-partition bias on ScalarE)
                nc.scalar.activation(
                    ot[:, m], ps[m], mybir.ActivationFunctionType.Identity,
                    bias=fsq_sb[:, m:m + 1],
                )
                # ot[m] += csq[col]  (per-column bias on VectorE, fp16)
                nc.vector.tensor_add(ot[:, m], ot[:, m], csq_sb[:, j])
                if j == NT - 1:
                    # Final n-tile: store per m-tile from the sync ring
                    # (idle by now) so the last store pipelines with the
                    # remaining evictions instead of waiting for all four.
                    nc.sync.dma_start(
                        out_r[:, m, j * 512:(j + 1) * 512], ot[:, m])
            if j < NT - 1:
                # Output on the second HWDGE ring (Activation engine):
                # separate FIFO from the input ring, so stores never queue
                # behind loads.
                nc.scalar.dma_start(out_r[:, :, j * 512:(j + 1) * 512], ot)


def _get_nc():
    global _NC
    if _NC is None:
        _NC = _build_bass()
    return _NC


def _prep_inputs(feat, centers):
    feat = np.asarray(feat, dtype=np.float32)
    centers = np.asarray(centers, dtype=np.float32)
    f8 = ml_dtypes.float8_e4m3

    cent_pad = np.zeros((C_PAD, FEAT_DIM), dtype=np.float32)
    cent_pad[:NUM_CLASSES] = centers
    # centsT_sw[j, p, ko*512 + c] = centers[j*512 + c, ko*128 + p]
    centsT_sw = np.ascontiguousarray(
        cent_pad.T.astype(f8).reshape(KO, P, NT, 512).transpose(2, 1, 0, 3)
    ).reshape(NT, P, KO * 512)

    csq_v = np.zeros(C_PAD, dtype=np.float32)
    csq_v[:NUM_CLASSES] = -0.5 * np.einsum("cd,cd->c", centers, centers)
    csq_sw = np.ascontiguousarray(np.broadcast_to(
        csq_v.astype(np.float16).reshape(NT // 4, 1, 4 * 512),
        (NT // 4, P, 4 * 512)))

    feat8 = feat.astype(f8)
    fsq_v = -0.5 * np.einsum("bd,bd->b", feat, feat)

    in_maps = []
    for i in range(N_CORES):
        r0 = i * B_PER
        # featT_sw[mt, p, ko*128 + m] = feat[r0 + mt*128 + m, ko*128 + p]
        featT_sw = np.ascontiguousarray(
            feat8[r0:r0 + B_PER].T.reshape(KO, P, MT, P).transpose(2, 1, 0, 3)
        ).reshape(MT, P, KO * P)
        fsq_mat = np.ascontiguousarray(
            fsq_v[r0:r0 + B_PER].reshape(MT, P).T)
        in_maps.append({
            "featT": featT_sw,
            "centsT": centsT_sw,
            "fsq": fsq_mat,
            "csq": csq_sw,
        })
    return in_maps


def _run(inputs, trace=False, trace_cores=None):
    from concourse import bass_utils

    nc = _get_nc()
    in_maps = _prep_inputs(inputs["feat"], inputs["centers"])
    res = bass_utils.run_bass_kernel_spmd(
        nc, in_maps, core_ids=list(range(N_CORES)), trace=trace,
        trace_cores=trace_cores,
    )
    full = np.concatenate(
        [np.asarray(res.results[i]["out"]) for i in range(N_CORES)], axis=0)
    return full[:, :NUM_CLASSES].astype(np.float32), res


def kernel(**inputs) -> np.ndarray:
    return _run(inputs)[0]


# revision 16
# speedup vs baseline: 1.1666x; 1.1060x over previous
"""LDA-loss logits kernel for Trainium2 (8 NeuronCores, SPMD).

Computes logits[b, c] = -0.5 * ||feat[b] - centers[c]||^2
                      = feat[b]·centers[c] - 0.5||feat[b]||^2 - 0.5||centers[c]||^2

Strategy (v3, fp8 DoubleRow):
  - Shard feat over batch: 4096 rows -> 512/core (4 m-tiles of 128), centers
    replicated.  Classes padded 10000 -> 10240 = 20 n-tiles of 512 so every
    matmul streams a full 512-wide moving operand (hides LDWEIGHTS).
  - Inputs quantized to fp8e4 on host; matmuls run perf_mode=DoubleRow
    (2 contraction chunks of 128 per pass -> ~1.8x bf16 column rate).  The
    squared-norm biases are host-precomputed fp32/fp16; fp8 error only
    touches the cross term (~6e-3 rel, tolerance 2e-2).
  - n-outer loop; center n-tiles stream in individually, and the first
    n-tile/feat are further split (per k-pair / per m-tile) so the first
    matmul issues after ~256KB of DMA instead of the full working set.
  - Eviction: ScalarE adds the per-row bias (fp32 PSUM -> fp16 SBUF),
    VectorE adds the per-column bias (fp16, 2x DVE rate).  The host upcasts
    the fp16 output after the gather (halves output HBM traffic).
  - All DMA on the sync HWDGE queue (SWDGE/gpsimd pays ~2us completion
    latency per store and a multi-us drain on the final ack); outputs are
    batched 4 m-tiles per store so the queue holds 20 output DMAs, not 80.
"""

import numpy as np
import ml_dtypes

BATCH = 4096
FEAT_DIM = 1024
NUM_CLASSES = 10000
N_CORES = 8
B_PER = BATCH // N_CORES            # 512 rows per core
P = 128
MT = B_PER // P                     # 4 output row tiles per core
KO = FEAT_DIM // P                  # 8 contraction chunks
KP = KO // 2                        # 4 DoubleRow chunk-pairs
NT = 20                             # n-tiles of 512
C_PAD = NT * 512                    # 10240 padded classes

_NC = None


def _build_bass():
    import concourse.mybir as mybir
    import concourse.tile as tile
    from concourse import bacc

    nc = bacc.Bacc("TRN2", target_bir_lowering=False, debug=False)

    featT = nc.dram_tensor("featT", [MT, P, KO * P], mybir.dt.float8e4,
                           kind="ExternalInput")
    centsT = nc.dram_tensor("centsT", [NT, P, KO * 512], mybir.dt.float8e4,
                            kind="ExternalInput")
    fsq = nc.dram_tensor("fsq", [P, MT], mybir.dt.float32, kind="ExternalInput")
    csq = nc.dram_tensor("csq", [NT // 4, P, 4 * 512], mybir.dt.float16,
                         kind="ExternalInput")
    out = nc.dram_tensor("out", [B_PER, C_PAD], mybir.dt.float16,
                         kind="ExternalOutput")

    with tile.TileContext(nc) as tc:
        _lda_tile_kernel(tc, featT.ap(), centsT.ap(), fsq.ap(), csq.ap(),
                         out.ap())
    nc.compile()
    return nc


def _lda_tile_kernel(tc, featT, centsT, fsq, csq, out):
    import concourse.mybir as mybir

    nc = tc.nc
    out_r = out.rearrange("(mo p) c -> p mo c", p=P)

    with (
        tc.tile_pool(name="big", bufs=1) as big,
        tc.tile_pool(name="consts", bufs=1) as consts,
        tc.tile_pool(name="ostage", bufs=6) as ostage,
        tc.tile_pool(name="psum", bufs=8, space="PSUM") as psum,
    ):
        cent_sb = big.tile([P, NT, KO, 512], mybir.dt.float8e4)
        feat_sb = big.tile([P, MT, KO, P], mybir.dt.float8e4)
        csq_sb = consts.tile([P, NT, 512], mybir.dt.float16)
        fsq_sb = consts.tile([P, MT], mybir.dt.float32)
        warm_sb = consts.tile([P, 192], mybir.dt.float8e4)

        # All input loads on the sync HWDGE queue in consumption order.  The
        # first matmul needs only feat m-tile 0 + the first k-pair of center
        # n-tile 0 (~256KB), so split those loads fine-grained; everything
        # later goes in n-tile-sized chunks that stay ahead of compute.
        c0 = centsT[0].rearrange("p (ko c) -> p ko c", ko=KO)
        nc.sync.dma_start(feat_sb[:, 0],
                          featT[0].rearrange("p (ko f) -> p ko f", ko=KO))
        nc.sync.dma_start(cent_sb[:, 0, 0:2], c0[:, 0:2])
        # feat m1-3 as one DMA: the whole batch lands before any of it is
        # consumed, and the freed dispatch slots pull cents0 kp1-3 and
        # cents1 earlier (cents1 as the 10th serial ~633ns dispatch arrived
        # ~0.4us after the j=1 matmuls wanted it).
        nc.sync.dma_start(
            feat_sb[:, 1:MT],
            featT[1:MT].rearrange("m p (ko f) -> p m ko f", ko=KO))
        for kp in range(1, KP):
            nc.sync.dma_start(cent_sb[:, 0, 2 * kp:2 * kp + 2],
                              c0[:, 2 * kp:2 * kp + 2])
        nc.sync.dma_start(fsq_sb[:], fsq)
        # Per-tile center loads: the tile framework signals completion
        # per-DMA, so coarser batches delay every tile in the batch to the
        # last byte and starve the matmul stream mid-flight (measured ~8us
        # of tensor gaps with 4-tile batches).  ~633ns of dispatch per DMA
        # is the cheaper side of that trade.
        nc.sync.dma_start(cent_sb[:, 1],
                          centsT[1].rearrange("p (ko c) -> p ko c", ko=KO))
        nc.sync.dma_start(cent_sb[:, 2],
                          centsT[2].rearrange("p (ko c) -> p ko c", ko=KO))
        nc.sync.dma_start(csq_sb[:, 0:4],
                          csq[0].rearrange("p (j c) -> p j c", j=4))
        for j in range(3, NT):
            nc.sync.dma_start(cent_sb[:, j],
                              centsT[j].rearrange("p (ko c) -> p ko c", ko=KO))
            if j % 4 == 1:
                b = j // 4
                nc.sync.dma_start(
                    csq_sb[:, 4 * b:4 * b + 4],
                    csq[b].rearrange("p (j c) -> p j c", j=4))

        # PE warm-up: ~6us of throwaway matmuls during the DMA prologue so
        # the HAM clock gate opens (1.2 -> 2.4 GHz takes ~3.4us of sustained
        # PE activity) before the first real matmul issues.
        nc.vector.memset(warm_sb[:], 0)
        warm_ps = psum.tile([P, 512], mybir.dt.float32, tag="ps", name="ps")
        for _ in range(56):
            nc.tensor.matmul(warm_ps[:, 0:64], warm_sb[:, 0:P],
                             warm_sb[:, P:P + 64], start=True, stop=True)

        for j in range(NT):
            ps = [psum.tile([P, 512], mybir.dt.float32, tag="ps", name="ps")
                  for _ in range(MT)]
            for kp in range(KP):
                for m in range(MT):
                    nc.tensor.matmul(
                        ps[m],
                        feat_sb[:, m, 2 * kp:2 * kp + 2, :],
                        cent_sb[:, j, 2 * kp:2 * kp + 2, :],
                        start=(kp == 0),
                        stop=(kp == KP - 1),
                        perf_mode=mybir.MatmulPerfMode.DoubleRow,
                    )
            ot = ostage.tile([P, MT, 512], mybir.dt.float16, tag="ot",
                             name="ot")
            for m in range(MT):
                # ot[m] = psum + fsq[row]  (per-partition bias on ScalarE)
                nc.scalar.activation(
                    ot[:, m], ps[m], mybir.ActivationFunctionType.Identity,
                    bias=fsq_sb[:, m:m + 1],
                )
                # ot[m] += csq[col]  (per-column bias on VectorE, fp16)
                nc.vector.tensor_add(ot[:, m], ot[:, m], csq_sb[:, j])
                if j == NT - 1:
                    # Final n-tile: store per m-tile from the sync ring
                    # (idle by now) so the last store pipelines with the
                    # remaining evictions instead of waiting for all four.
                    nc.sync.dma_start(
                        out_r[:, m, j * 512:(j + 1) * 512], ot[:, m])
            if j < NT - 1:
                # Output on the second HWDGE ring (Activation engine):
                # separate FIFO from the input ring, so stores never queue
                # behind loads.
                nc.scalar.dma_start(out_r[:, :, j * 512:(j + 1) * 512], ot)


def _get_nc():
    global _NC
    if _NC is None:
        _NC = _build_bass()
    return _NC


def _prep_inputs(feat, centers):
    feat = np.asarray(feat, dtype=np.float32)
    centers = np.asarray(centers, dtype=np.float32)
    f8 = ml_dtypes.float8_e4m3

    cent_pad = np.zeros((C_PAD, FEAT_DIM), dtype=np.float32)
    cent_pad[:NUM_CLASSES] = centers
    # centsT_sw[j, p, ko*512 + c] = centers[j*512 + c, ko*128 + p]
    centsT_sw = np.ascontiguousarray(
        cent_pad.T.astype(f8).reshape(KO, P, NT, 512).transpose(2, 1, 0, 3)
    ).reshape(NT, P, KO * 512)

    csq_v = np.zeros(C_PAD, dtype=np.float32)
    csq_v[:NUM_CLASSES] = -0.5 * np.einsum("cd,cd->c", centers, centers)
    csq_sw = np.ascontiguousarray(np.broadcast_to(
        csq_v.astype(np.float16).reshape(NT // 4, 1, 4 * 512),
        (NT // 4, P, 4 * 512)))

    feat8 = feat.astype(f8)
    fsq_v = -0.5 * np.einsum("bd,bd->b", feat, feat)

    in_maps = []
    for i in range(N_CORES):
        r0 = i * B_PER
        # featT_sw[mt, p, ko*128 + m] = feat[r0 + mt*128 + m, ko*128 + p]
        featT_sw = np.ascontiguousarray(
            feat8[r0:r0 + B_PER].T.reshape(KO, P, MT, P).transpose(2, 1, 0, 3)
        ).reshape(MT, P, KO * P)
        fsq_mat = np.ascontiguousarray(
            fsq_v[r0:r0 + B_PER].reshape(MT, P).T)
        in_maps.append({
            "featT": featT_sw,
            "centsT": centsT_sw,
            "fsq": fsq_mat,
            "csq": csq_sw,
        })
    return in_maps


def _run(inputs, trace=False, trace_cores=None):
    from concourse import bass_utils

    nc = _get_nc()
    in_maps = _prep_inputs(inputs["feat"], inputs["centers"])
    res = bass_utils.run_bass_kernel_spmd(
        nc, in_maps, core_ids=list(range(N_CORES)), trace=trace,
        trace_cores=trace_cores,
    )
    full = np.concatenate(
        [np.asarray(res.results[i]["out"]) for i in range(N_CORES)], axis=0)
    return full[:, :NUM_CLASSES].astype(np.float32), res


def kernel(**inputs) -> np.ndarray:
    return _run(inputs)[0]
